# revision 1
# baseline (speedup 1.0000x reference)
"""MoE downsample kernel for 8 TRN2 NeuronCores.

Strategy: data-parallel over batch (2 samples per core). Each strided dilated
conv is decomposed into k*k "tap" matmuls (lhsT = [Cin=64, Cout=64] weight
slice, rhs = strided view of the zero-padded input image) accumulated in PSUM
over 512-pixel output chunks. Samples map to PE-array row halves (partitions
0-63 / 64-127) and two balanced expert queues map to PE col halves, so four
64x64 tile_position matmuls run concurrently (full 128x128 array).
BN + conv-bias + GELU are fused into the ScalarE PSUM eviction.
Gating (tiny: 16x64x4 matvec + softmax + top2) and final top-2 assembly run
on host.
"""

import numpy as np
import ml_dtypes

KS = [3, 5, 7, 9]
DS = [1, 2, 3, 4]
BN_EPS = 1e-5
B, CIN, H, W = 16, 64, 256, 256
CE = 64
PAD = 16          # left/top pad (max |offset|); right/bottom needs 15
HP = WP = PAD + 256 + 15   # 287
HO = WO = 128
NCORES = 8
SPC = 2           # samples per core
NTAPS = sum(k * k for k in KS)  # 164
CHUNK_ROWS = 4    # output rows per 512-px chunk
NCHUNKS = HO // CHUNK_ROWS      # 32

# tap slot base per expert
_SLOT_BASE = np.cumsum([0] + [k * k for k in KS]).tolist()

# queue split: col0 = experts [0,1,2] (83 taps), col1 = [3] (81 taps)
COL_EXPERTS = [[0, 1, 2], [3]]

_COMPILED = None


def _tap_offsets(e):
    """Yield (slot, row_off, col_off) in padded coords for expert e, tap (u,v)."""
    k, d = KS[e], DS[e]
    pad = d * (k - 1) // 2
    for u in range(k):
        for v in range(k):
            slot = _SLOT_BASE[e] + u * k + v
            yield slot, d * u - pad + PAD, d * v - pad + PAD


def _build_program():
    import concourse.bass as bass  # noqa: F401
    import concourse.mybir as mybir
    import concourse.tile as tile
    from concourse import bacc
    from contextlib import ExitStack

    dt = mybir.dt
    nc = bacc.Bacc("TRN2", target_bir_lowering=False, debug=False,
                   num_devices=NCORES)
    xpad = nc.dram_tensor("xpad", [SPC, CIN, HP, WP], dt.bfloat16,
                          kind="ExternalInput")
    wt = nc.dram_tensor("wt", [CIN, NTAPS, CE], dt.bfloat16,
                        kind="ExternalInput")
    bnp = nc.dram_tensor("bnp", [CE, 4, 2], dt.float32, kind="ExternalInput")
    out = nc.dram_tensor("out", [SPC, 4, CE, HO, WO], dt.float32,
                         kind="ExternalOutput")

    with tile.TileContext(nc) as tc:
        with ExitStack() as ctx:
            consts = ctx.enter_context(tc.tile_pool(name="consts", bufs=1))
            stage_pool = ctx.enter_context(tc.tile_pool(name="st", bufs=8))

            # ---- load constants / inputs into SBUF ----
            # first x stripe (rows needed by chunk 0) goes out first so the
            # PE can start as early as possible; weights ride alongside.
            wtile = consts.tile([128, NTAPS, CE], dt.bfloat16)
            bntile = consts.tile([128, 4, 2], dt.float32)
            xtile = consts.tile([128, HP, WP], dt.bfloat16)
            bounds = [0, 44] + [44 + ((HP - 44) * p) // 14
                                for p in range(1, 15)]
            for s in range(SPC):
                nc.gpsimd.dma_start(
                    out=xtile[s * 64:(s + 1) * 64, 0:44, :],
                    in_=xpad[s, :, 0:44, :])
            for half in range(2):
                p0 = half * 64
                nc.gpsimd.dma_start(out=wtile[p0:p0 + 64, :, :], in_=wt.ap())
                nc.gpsimd.dma_start(out=bntile[p0:p0 + 64, :, :], in_=bnp.ap())
            for piece in range(1, len(bounds) - 1):
                for s in range(SPC):
                    r0, r1 = bounds[piece], bounds[piece + 1]
                    nc.gpsimd.dma_start(
                        out=xtile[s * 64:(s + 1) * 64, r0:r1, :],
                        in_=xpad[s, :, r0:r1, :])

            psum_pool = ctx.enter_context(
                tc.tile_pool(name="ps", bufs=8, space="PSUM"))

            # ---- main loop: 32 chunks x (2 samples x 2 col-queues) ----
            def queue_events(s, col, r):
                p0 = s * 64           # rhs/lhsT partition base (PE rows)
                q0 = col * 64         # psum/out partition base (PE cols)
                i0 = r * CHUNK_ROWS   # first output row
                for e in COL_EXPERTS[col]:
                    ps = psum_pool.tile([128, 512], dt.float32)
                    taps = list(_tap_offsets(e))
                    for t, (slot, ro, co) in enumerate(taps):
                        r_lo = 2 * i0 + ro
                        rhs = xtile[p0:p0 + 64,
                                    r_lo:r_lo + 2 * CHUNK_ROWS - 1:2,
                                    co:co + 2 * WO - 1:2]
                        lhsT = wtile[p0:p0 + 64, slot, :]
                        psv = ps[q0:q0 + 64, :]
                        first = t == 0
                        last = t == len(taps) - 1

                        def mm(rhs=rhs, lhsT=lhsT, psv=psv, first=first,
                               last=last, p0=p0, q0=q0):
                            nc.tensor.matmul(psv, lhsT, rhs, start=first,
                                             stop=last,
                                             tile_position=(p0, q0))
                        yield ("mm", mm)

                    def evict(ps=ps, s=s, e=e, i0=i0, q0=q0):
                        st = stage_pool.tile([128, CHUNK_ROWS, WO],
                                             dt.float32)
                        nc.scalar.activation(
                            st[q0:q0 + 64, :, :],
                            ps[q0:q0 + 64, :].rearrange(
                                "p (a b) -> p a b", a=CHUNK_ROWS),
                            mybir.ActivationFunctionType.Gelu,
                            scale=bntile[q0:q0 + 64, e, 0:1],
                            bias=bntile[q0:q0 + 64, e, 1:2])
                        nc.sync.dma_start(
                            out=out[s, e, :, i0:i0 + CHUNK_ROWS, :],
                            in_=st[q0:q0 + 64, :, :])
                    yield ("evict", evict)

            for r in range(NCHUNKS):
                queues = [queue_events(s, col, r)
                          for s in range(SPC) for col in range(2)]
                live = list(queues)
                while live:
                    nxt = []
                    for q in live:
                        ev = next(q, None)
                        if ev is None:
                            continue
                        ev[1]()
                        nxt.append(q)
                    live = nxt

    nc.compile()
    return nc


def _get_program():
    global _COMPILED
    if _COMPILED is None:
        _COMPILED = _build_program()
    return _COMPILED


def _host_gate(x, gate_w, gate_b):
    """Replicate reference gating in numpy (f64 pooling for robustness)."""
    pooled = x.astype(np.float64).mean(axis=(2, 3)).astype(np.float32)
    logits = pooled @ gate_w.T.astype(np.float32) + gate_b
    z = logits - logits.max(axis=1, keepdims=True)
    ez = np.exp(z.astype(np.float32))
    gates = ez / ez.sum(axis=1, keepdims=True)
    idx = np.argsort(-gates, axis=1, kind="stable")[:, :2]
    wsel = np.take_along_axis(gates, idx, axis=1)
    wsel = wsel / (wsel.sum(axis=1, keepdims=True) + 1e-8)
    return idx, wsel.astype(np.float32)


def _prep_inputs(x, ws, bs, bn_scale, bn_bias, bn_mean, bn_var):
    bf16 = ml_dtypes.bfloat16
    # padded bf16 images, per core
    xpad = np.zeros((B, CIN, HP, WP), dtype=bf16)
    xpad[:, :, PAD:PAD + H, PAD:PAD + W] = x.astype(bf16)

    # transposed weights, DMA-friendly layout [CIN, NTAPS, CE]
    wt = np.empty((CIN, NTAPS, CE), dtype=bf16)
    for e in range(4):
        k = KS[e]
        w = ws[e].astype(np.float32)  # [CE, CIN, k, k]
        # [CE, CIN, k, k] -> [CIN, k*k, CE]
        wt[:, _SLOT_BASE[e]:_SLOT_BASE[e] + k * k, :] = (
            w.transpose(1, 2, 3, 0).reshape(CIN, k * k, CE).astype(bf16))

    # folded BN: z = conv*scale + shift ; scale = bn_scale*rsqrt(var+eps),
    # shift = conv_bias*scale + bn_bias - mean*scale
    inv = (bn_scale / np.sqrt(bn_var + BN_EPS)).astype(np.float32)
    shift = (np.stack(bs) * inv + bn_bias - bn_mean * inv).astype(np.float32)
    bnp = np.stack([inv, shift], axis=1)  # [4, 2, CE]
    bnp = np.ascontiguousarray(bnp.transpose(2, 0, 1))  # [CE, 4, 2]
    return xpad, wt, bnp


def run(inputs, trace=False):
    from concourse import bass_utils

    x = np.asarray(inputs["x"], dtype=np.float32)
    ws = [np.asarray(inputs[f"w{i}"], dtype=np.float32) for i in range(4)]
    bs = [np.asarray(inputs[f"b{i}"], dtype=np.float32) for i in range(4)]
    bn_scale = np.asarray(inputs["bn_scale"], dtype=np.float32)
    bn_bias = np.asarray(inputs["bn_bias"], dtype=np.float32)
    bn_mean = np.asarray(inputs["bn_mean"], dtype=np.float32)
    bn_var = np.asarray(inputs["bn_var"], dtype=np.float32)
    gate_w = np.asarray(inputs["gate_w"], dtype=np.float32)
    gate_b = np.asarray(inputs["gate_b"], dtype=np.float32)

    nc = _get_program()
    xpad, wt, bnp = _prep_inputs(x, ws, bs, bn_scale, bn_bias, bn_mean,
                                 bn_var)
    in_maps = []
    for c in range(NCORES):
        in_maps.append({
            "xpad": xpad[c * SPC:(c + 1) * SPC],
            "wt": wt,
            "bnp": bnp,
        })
    res = bass_utils.run_bass_kernel_spmd(
        nc, in_maps, core_ids=list(range(NCORES)), trace=trace)

    # assemble: E[b, e] for all experts, then host top-2 select/scale/concat
    E = np.concatenate([res.results[c]["out"] for c in range(NCORES)],
                       axis=0)  # [B, 4, CE, HO, WO]
    idx, wsel = _host_gate(x, gate_w, gate_b)
    outf = np.empty((B, 2 * CE, HO, WO), dtype=np.float32)
    for b in range(B):
        outf[b, :CE] = E[b, idx[b, 0]] * wsel[b, 0]
        outf[b, CE:] = E[b, idx[b, 1]] * wsel[b, 1]
    return outf, res


def kernel(**inputs):
    outf, _ = run(inputs, trace=False)
    return outf



# revision 6
# speedup vs baseline: 1.0367x; 1.0367x over previous
"""MoE downsample kernel for 8 TRN2 NeuronCores — top-2 sparse version.

Host computes the gate (cheap 16x64x4 matvec) first, so the device only
computes each sample's two selected experts (~half the dense MACs). The
SPMD constraint (one program on all 8 cores) is satisfied by an
expert-major uniform schedule parameterized only by the per-expert
selection counts n_e: for expert e every row-half instance (8 cores x 2
PE row-halves = 16 instances) runs an identical pattern of chunk-runs
(run lengths = binary decomposition of 2*n_e into {8,4,2}); the host
gathers the right (sample, chunk-range) input slab into each run's slot
and scatters the outputs back.

Per run of L chunks (chunk = 4 output rows = 512 px): the two PE column
halves process L/2 chunks each, tap-outer loop so one LDWEIGHTS per
(tap, quadrant) covers L/2 matmuls (trailing matmuls set ldweights=False
to reuse the loaded stationary operand). BN + conv-bias + GELU fused
into the ScalarE PSUM eviction; outputs written bf16; gate weighting and
top-2 concat on host.
"""

import numpy as np
import ml_dtypes

KS = [3, 5, 7, 9]
DS = [1, 2, 3, 4]
BN_EPS = 1e-5
B, CIN, H, W = 16, 64, 256, 256
CE = 64
PAD = 16
HP = WP = PAD + 256 + 15   # 287
HO = WO = 128
NCORES = 8
NTAPS = sum(k * k for k in KS)  # 164
NCHUNKS = 32                    # 4 output rows per chunk
EXPERT_ORDER = [3, 2, 1, 0]     # heavy taps first (prefetch headroom)
USE_LDW_SKIP = True

_SLOT_BASE = np.cumsum([0] + [k * k for k in KS]).tolist()

_COMPILED = {}


def _tap_offsets(e):
    """(slot, row_off, col_off) in padded slab coords for expert e."""
    k, d = KS[e], DS[e]
    pad = d * (k - 1) // 2
    for u in range(k):
        for v in range(k):
            slot = _SLOT_BASE[e] + u * k + v
            yield slot, d * u - pad + PAD, d * v - pad + PAD


def _pattern(two_n):
    """Run lengths (each in {2,4,8}, smallest first) summing to 2*n_e."""
    out = []
    if two_n & 2:
        out.append(2)
    if two_n & 4:
        out.append(4)
    out += [8] * ((two_n - sum(out)) // 8)
    assert sum(out) == two_n
    return out


def _gate(x, gate_w, gate_b):
    pooled = x.astype(np.float64).mean(axis=(2, 3)).astype(np.float32)
    logits = pooled @ gate_w.T.astype(np.float32) + gate_b
    z = logits - logits.max(axis=1, keepdims=True)
    ez = np.exp(z.astype(np.float32))
    gates = ez / ez.sum(axis=1, keepdims=True)
    idx = np.argsort(-gates, axis=1, kind="stable")[:, :2]
    wsel = np.take_along_axis(gates, idx, axis=1)
    wsel = wsel / (wsel.sum(axis=1, keepdims=True) + 1e-8)
    return idx, wsel.astype(np.float32)


def _plan(idx):
    """Build the uniform schedule + per-instance run assignment.

    Returns (key, patterns, runseq, units, assign):
      key      — cache key for the compiled program (depends on n only)
      patterns — {e: [run lengths]}
      runseq   — [(e, L, rowoff, rows)] in program order
      units    — [(e, pos, rh, j, L2)] out-unit order (per core)
      assign   — {(instance 0..15, e, pos): (sample, c0)}
    """
    n = [0, 0, 0, 0]
    samples_e = {e: [] for e in range(4)}
    for s in range(B):
        for e in idx[s]:
            n[e] += 1
            samples_e[e].append(s)
    patterns = {e: _pattern(2 * n[e]) for e in range(4)}
    key = tuple(n)

    # global run inventory per (e, L): cut samples into within-sample runs
    assign = {}
    for e in EXPERT_ORDER:
        pat = patterns[e]
        need = {}
        for L in pat:
            need[L] = need.get(L, 0) + 16
        cursor = 0
        inventory = {L: [] for L in need}
        for L in sorted(need, reverse=True):
            ns = need[L] * L // NCHUNKS     # samples consumed at this L
            assert ns * NCHUNKS == need[L] * L
            for _ in range(ns):
                s = samples_e[e][cursor]
                cursor += 1
                for c0 in range(0, NCHUNKS, L):
                    inventory[L].append((s, c0))
        assert cursor == n[e], (e, cursor, n[e])
        ptr = {L: 0 for L in need}
        for pos, L in enumerate(pat):
            for i in range(16):
                assign[(i, e, pos)] = inventory[L][ptr[L]]
                ptr[L] += 1
        for L in need:
            assert ptr[L] == len(inventory[L])

    runseq = []
    units = []
    rowoff = 0
    for e in EXPERT_ORDER:
        for pos, L in enumerate(patterns[e]):
            rows = 8 * L + 31
            runseq.append((e, L, rowoff, rows))
            rowoff += rows
            L2 = L // 2
            for rh in range(2):
                for j in range(L2):
                    units.append((e, pos, rh, j, L2))
    return key, patterns, runseq, units, assign


def _build_program(runseq):
    import concourse.bass as bass  # noqa: F401
    import concourse.mybir as mybir
    import concourse.tile as tile
    from concourse import bacc
    from contextlib import ExitStack

    dt = mybir.dt
    totrows = sum(r[3] for r in runseq)
    nunits = sum(2 * (r[1] // 2) for r in runseq)

    nc = bacc.Bacc("TRN2", target_bir_lowering=False, debug=False,
                   num_devices=NCORES)
    xin0 = nc.dram_tensor("xin0", [CIN, totrows, WP], dt.bfloat16,
                          kind="ExternalInput")
    xin1 = nc.dram_tensor("xin1", [CIN, totrows, WP], dt.bfloat16,
                          kind="ExternalInput")
    wt = nc.dram_tensor("wt", [CIN, NTAPS, CE], dt.bfloat16,
                        kind="ExternalInput")
    bnp = nc.dram_tensor("bnp", [CE, 4, 2], dt.float32, kind="ExternalInput")
    out = nc.dram_tensor("out", [128, nunits, 512], dt.bfloat16,
                         kind="ExternalOutput")

    with tile.TileContext(nc) as tc:
        with ExitStack() as ctx:
            consts = ctx.enter_context(tc.tile_pool(name="consts", bufs=1))
            slab_pool = ctx.enter_context(tc.tile_pool(name="slab", bufs=3))
            stage_pool = ctx.enter_context(tc.tile_pool(name="st", bufs=8))
            psum_pool = ctx.enter_context(
                tc.tile_pool(name="ps", bufs=1, space="PSUM"))

            wtile = consts.tile([128, NTAPS, CE], dt.bfloat16)
            bntile = consts.tile([128, 4, 2], dt.float32)
            for half in range(2):
                p0 = half * 64
                nc.gpsimd.dma_start(out=wtile[p0:p0 + 64, :, :], in_=wt.ap())
                nc.gpsimd.dma_start(out=bntile[p0:p0 + 64, :, :], in_=bnp.ap())

            # HAM warmup: ~7us of dummy matmuls while the first slab DMAs.
            warm_ps = psum_pool.tile([128, 512], dt.float32, name="psb_0_0")
            for _ in range(16):
                nc.tensor.matmul(warm_ps[0:64, :], wtile[0:64, 0, :],
                                 wtile[0:64, 0:8, :], start=True, stop=True,
                                 tile_position=(0, 0))

            unit = 0
            for e, L, rowoff, rows in runseq:
                taps = list(_tap_offsets(e))
                T = len(taps)
                L2 = L // 2
                st = slab_pool.tile([128, 8 * 8 + 31, WP], dt.bfloat16,
                                    name="st")
                nc.gpsimd.dma_start(out=st[0:64, 0:rows, :],
                                    in_=xin0[:, rowoff:rowoff + rows, :])
                nc.gpsimd.dma_start(out=st[64:128, 0:rows, :],
                                    in_=xin1[:, rowoff:rowoff + rows, :])
                ps = [[psum_pool.tile([128, 512], dt.float32,
                                      name=f"psb_{rh}_{j}")
                       for j in range(L2)] for rh in range(2)]
                for t, (slot, ro, co) in enumerate(taps):
                    first = t == 0
                    last = t == T - 1
                    for rh in range(2):
                        p0 = rh * 64
                        lhsT = wtile[p0:p0 + 64, slot, :]
                        for col in range(2):
                            q0 = col * 64
                            for j in range(L2):
                                cc = col * L2 + j
                                r_lo = 8 * cc + ro
                                rhs = st[p0:p0 + 64,
                                         r_lo:r_lo + 7:2,
                                         co:co + 2 * WO - 1:2]
                                mm = nc.tensor.matmul(
                                    ps[rh][j][q0:q0 + 64, :], lhsT, rhs,
                                    start=first, stop=last,
                                    tile_position=(p0, q0))
                                if USE_LDW_SKIP and j > 0:
                                    mm.ldweights = False
                for rh in range(2):
                    for j in range(L2):
                        stg = stage_pool.tile([128, 512], dt.bfloat16)
                        nc.scalar.activation(
                            stg, ps[rh][j],
                            mybir.ActivationFunctionType.Gelu,
                            scale=bntile[:, e, 0:1],
                            bias=bntile[:, e, 1:2])
                        nc.sync.dma_start(out=out[:, unit, :], in_=stg)
                        unit += 1
            assert unit == nunits

    nc.compile()
    return nc


def _get_program(key, runseq):
    if key not in _COMPILED:
        _COMPILED[key] = _build_program(runseq)
    return _COMPILED[key]


def _prep_weights(ws, bs, bn_scale, bn_bias, bn_mean, bn_var):
    bf16 = ml_dtypes.bfloat16
    wt = np.empty((CIN, NTAPS, CE), dtype=bf16)
    for e in range(4):
        k = KS[e]
        w = ws[e].astype(np.float32)  # [CE, CIN, k, k]
        wt[:, _SLOT_BASE[e]:_SLOT_BASE[e] + k * k, :] = (
            w.transpose(1, 2, 3, 0).reshape(CIN, k * k, CE).astype(bf16))
    inv = (bn_scale / np.sqrt(bn_var + BN_EPS)).astype(np.float32)
    shift = (np.stack(bs) * inv + bn_bias - bn_mean * inv).astype(np.float32)
    bnp = np.stack([inv, shift], axis=1)              # [4, 2, CE]
    bnp = np.ascontiguousarray(bnp.transpose(2, 0, 1))  # [CE, 4, 2]
    return wt, bnp


def run(inputs, trace=False):
    from concourse import bass_utils

    x = np.asarray(inputs["x"], dtype=np.float32)
    ws = [np.asarray(inputs[f"w{i}"], dtype=np.float32) for i in range(4)]
    bs = [np.asarray(inputs[f"b{i}"], dtype=np.float32) for i in range(4)]
    bn_scale = np.asarray(inputs["bn_scale"], dtype=np.float32)
    bn_bias = np.asarray(inputs["bn_bias"], dtype=np.float32)
    bn_mean = np.asarray(inputs["bn_mean"], dtype=np.float32)
    bn_var = np.asarray(inputs["bn_var"], dtype=np.float32)
    gate_w = np.asarray(inputs["gate_w"], dtype=np.float32)
    gate_b = np.asarray(inputs["gate_b"], dtype=np.float32)

    idx, wsel = _gate(x, gate_w, gate_b)
    key, patterns, runseq, units, assign = _plan(idx)
    nc = _get_program(key, runseq)
    wt, bnp = _prep_weights(ws, bs, bn_scale, bn_bias, bn_mean, bn_var)

    bf16 = ml_dtypes.bfloat16
    xpad = np.zeros((B, CIN, HP, WP), dtype=bf16)
    xpad[:, :, PAD:PAD + H, PAD:PAD + W] = x.astype(bf16)

    totrows = sum(r[3] for r in runseq)
    in_maps = []
    for c in range(NCORES):
        xin = np.empty((2, CIN, totrows, WP), dtype=bf16)
        for rh in range(2):
            i = c * 2 + rh
            pos_ctr = {}
            for e, L, rowoff, rows in runseq:
                pos = pos_ctr.get(e, 0)
                pos_ctr[e] = pos + 1
                s, c0 = assign[(i, e, pos)]
                xin[rh, :, rowoff:rowoff + rows, :] = (
                    xpad[s, :, 8 * c0:8 * c0 + rows, :])
        in_maps.append({"xin0": xin[0], "xin1": xin[1],
                        "wt": wt, "bnp": bnp})

    res = bass_utils.run_bass_kernel_spmd(
        nc, in_maps, core_ids=list(range(NCORES)), trace=trace)

    # scatter device outputs -> (sample, rank) feature maps, weight, concat
    rank = {}
    for s in range(B):
        rank[(s, idx[s, 0])] = 0
        rank[(s, idx[s, 1])] = 1
    E = np.zeros((B, 2, CE, HO, WO), dtype=np.float32)
    # per-(e) position counters replicated per core (same unit order)
    for c in range(NCORES):
        o = res.results[c]["out"]  # [128, nunits, 512] bf16
        of = o.astype(np.float32)
        for u, (e, pos, rh, j, L2) in enumerate(units):
            i = c * 2 + rh
            s, c0 = assign[(i, e, pos)]
            r = rank[(s, e)]
            ca, cb = c0 + j, c0 + L2 + j
            E[s, r, :, 4 * ca:4 * ca + 4, :] = of[0:64, u].reshape(64, 4, WO)
            E[s, r, :, 4 * cb:4 * cb + 4, :] = of[64:128, u].reshape(64, 4, WO)
    outf = (E * wsel[:, :, None, None, None]).reshape(B, 2 * CE, HO, WO)
    return np.ascontiguousarray(outf), res


def kernel(**inputs):
    outf, _ = run(inputs, trace=False)
    return outf


# revision 7
# speedup vs baseline: 1.2471x; 1.2029x over previous
"""MoE downsample kernel for 8 TRN2 NeuronCores — top-2 sparse version.

Host computes the gate (cheap 16x64x4 matvec) first, so the device only
computes each sample's two selected experts (~half the dense MACs). The
SPMD constraint (one program on all 8 cores) is satisfied by an
expert-major uniform schedule parameterized only by the per-expert
selection counts n_e: for expert e every row-half instance (8 cores x 2
PE row-halves = 16 instances) runs an identical pattern of chunk-runs
(run lengths = binary decomposition of 2*n_e into {8,4,2}); the host
gathers the right (sample, chunk-range) input slab into each run's slot
and scatters the outputs back.

Per run of L chunks (chunk = 4 output rows = 512 px): the two PE column
halves process L/2 chunks each, tap-outer loop so one LDWEIGHTS per
(tap, quadrant) covers L/2 matmuls (trailing matmuls set ldweights=False
to reuse the loaded stationary operand). BN + conv-bias + GELU fused
into the ScalarE PSUM eviction; outputs written bf16; gate weighting and
top-2 concat on host.
"""

import numpy as np
import ml_dtypes

KS = [3, 5, 7, 9]
DS = [1, 2, 3, 4]
BN_EPS = 1e-5
B, CIN, H, W = 16, 64, 256, 256
CE = 64
PAD = 16
HP = WP = PAD + 256 + 15   # 287
HO = WO = 128
NCORES = 8
NTAPS = sum(k * k for k in KS)  # 164
NCHUNKS = 32                    # 4 output rows per chunk
EXPERT_ORDER = [3, 2, 1, 0]     # heavy taps first (prefetch headroom)
USE_LDW_SKIP = True

_SLOT_BASE = np.cumsum([0] + [k * k for k in KS]).tolist()

_COMPILED = {}


def _tap_offsets(e):
    """(slot, row_off, col_off) in padded slab coords for expert e."""
    k, d = KS[e], DS[e]
    pad = d * (k - 1) // 2
    for u in range(k):
        for v in range(k):
            slot = _SLOT_BASE[e] + u * k + v
            yield slot, d * u - pad + PAD, d * v - pad + PAD


def _pattern(two_n):
    """Run lengths (each in {2,4,8}, smallest first) summing to 2*n_e."""
    out = []
    if two_n & 2:
        out.append(2)
    if two_n & 4:
        out.append(4)
    out += [8] * ((two_n - sum(out)) // 8)
    assert sum(out) == two_n
    return out


def _gate(x, gate_w, gate_b):
    pooled = x.astype(np.float64).mean(axis=(2, 3)).astype(np.float32)
    logits = pooled @ gate_w.T.astype(np.float32) + gate_b
    z = logits - logits.max(axis=1, keepdims=True)
    ez = np.exp(z.astype(np.float32))
    gates = ez / ez.sum(axis=1, keepdims=True)
    idx = np.argsort(-gates, axis=1, kind="stable")[:, :2]
    wsel = np.take_along_axis(gates, idx, axis=1)
    wsel = wsel / (wsel.sum(axis=1, keepdims=True) + 1e-8)
    return idx, wsel.astype(np.float32)


def _plan(idx):
    """Build the uniform schedule + per-instance run assignment.

    Returns (key, patterns, runseq, units, assign):
      key      — cache key for the compiled program (depends on n only)
      patterns — {e: [run lengths]}
      runseq   — [(e, L, rowoff, rows)] in program order
      units    — [(e, pos, rh, j, L2)] out-unit order (per core)
      assign   — {(instance 0..15, e, pos): (sample, c0)}
    """
    n = [0, 0, 0, 0]
    samples_e = {e: [] for e in range(4)}
    for s in range(B):
        for e in idx[s]:
            n[e] += 1
            samples_e[e].append(s)
    patterns = {e: _pattern(2 * n[e]) for e in range(4)}
    key = tuple(n)

    # global run inventory per (e, L): cut samples into within-sample runs
    assign = {}
    for e in EXPERT_ORDER:
        pat = patterns[e]
        need = {}
        for L in pat:
            need[L] = need.get(L, 0) + 16
        cursor = 0
        inventory = {L: [] for L in need}
        for L in sorted(need, reverse=True):
            ns = need[L] * L // NCHUNKS     # samples consumed at this L
            assert ns * NCHUNKS == need[L] * L
            for _ in range(ns):
                s = samples_e[e][cursor]
                cursor += 1
                for c0 in range(0, NCHUNKS, L):
                    inventory[L].append((s, c0))
        assert cursor == n[e], (e, cursor, n[e])
        ptr = {L: 0 for L in need}
        for pos, L in enumerate(pat):
            for i in range(16):
                assign[(i, e, pos)] = inventory[L][ptr[L]]
                ptr[L] += 1
        for L in need:
            assert ptr[L] == len(inventory[L])

    runseq = []
    units = []
    rowoff = 0
    for e in EXPERT_ORDER:
        for pos, L in enumerate(patterns[e]):
            rows = 8 * L + 31
            runseq.append((e, L, rowoff, rows))
            rowoff += rows
            L2 = L // 2
            for rh in range(2):
                for j in range(L2):
                    units.append((e, pos, rh, j, L2))
    return key, patterns, runseq, units, assign


def _build_program(runseq):
    import concourse.bass as bass  # noqa: F401
    import concourse.mybir as mybir
    import concourse.tile as tile
    from concourse import bacc
    from contextlib import ExitStack

    dt = mybir.dt
    totrows = sum(r[3] for r in runseq)
    nunits = sum(2 * (r[1] // 2) for r in runseq)

    nc = bacc.Bacc("TRN2", target_bir_lowering=False, debug=False,
                   num_devices=NCORES)
    xin0 = nc.dram_tensor("xin0", [CIN, totrows, WP], dt.bfloat16,
                          kind="ExternalInput")
    xin1 = nc.dram_tensor("xin1", [CIN, totrows, WP], dt.bfloat16,
                          kind="ExternalInput")
    wt = nc.dram_tensor("wt", [CIN, NTAPS, CE], dt.bfloat16,
                        kind="ExternalInput")
    bnp = nc.dram_tensor("bnp", [CE, 4, 2], dt.float32, kind="ExternalInput")
    out = nc.dram_tensor("out", [128, nunits, 512], dt.bfloat16,
                         kind="ExternalOutput")

    with tile.TileContext(nc) as tc:
        with ExitStack() as ctx:
            consts = ctx.enter_context(tc.tile_pool(name="consts", bufs=1))
            slab_pool = ctx.enter_context(tc.tile_pool(name="slab", bufs=3))
            stage_pool = ctx.enter_context(tc.tile_pool(name="st", bufs=8))
            psum_pool = ctx.enter_context(
                tc.tile_pool(name="ps", bufs=1, space="PSUM"))

            wtile = consts.tile([128, NTAPS, CE], dt.bfloat16)
            bntile = consts.tile([128, 4, 2], dt.float32)
            for half in range(2):
                p0 = half * 64
                nc.gpsimd.dma_start(out=wtile[p0:p0 + 64, :, :], in_=wt.ap())
                nc.gpsimd.dma_start(out=bntile[p0:p0 + 64, :, :], in_=bnp.ap())

            # HAM warmup: ~7us of dummy matmuls while the first slab DMAs.
            warm_ps = psum_pool.tile([128, 512], dt.float32, name="psb_0_0")
            for _ in range(16):
                nc.tensor.matmul(warm_ps[0:64, :], wtile[0:64, 0, :],
                                 wtile[0:64, 0:8, :], start=True, stop=True,
                                 tile_position=(0, 0))

            unit = 0
            for e, L, rowoff, rows in runseq:
                taps = list(_tap_offsets(e))
                T = len(taps)
                L2 = L // 2
                st = slab_pool.tile([128, 8 * 8 + 31, WP], dt.bfloat16,
                                    name="st")
                nc.gpsimd.dma_start(out=st[0:64, 0:rows, :],
                                    in_=xin0[:, rowoff:rowoff + rows, :])
                nc.gpsimd.dma_start(out=st[64:128, 0:rows, :],
                                    in_=xin1[:, rowoff:rowoff + rows, :])
                ps = [[psum_pool.tile([128, 512], dt.float32,
                                      name=f"psb_{rh}_{j}")
                       for j in range(L2)] for rh in range(2)]
                for t, (slot, ro, co) in enumerate(taps):
                    first = t == 0
                    last = t == T - 1
                    # chunk-index outer, quadrant inner: the PE queue is
                    # in-order, so consecutive MMs must hit different
                    # quadrants to keep all four streaming concurrently.
                    for j in range(L2):
                        for rh in range(2):
                            p0 = rh * 64
                            lhsT = wtile[p0:p0 + 64, slot, :]
                            for col in range(2):
                                q0 = col * 64
                                cc = col * L2 + j
                                r_lo = 8 * cc + ro
                                rhs = st[p0:p0 + 64,
                                         r_lo:r_lo + 7:2,
                                         co:co + 2 * WO - 1:2]
                                mm = nc.tensor.matmul(
                                    ps[rh][j][q0:q0 + 64, :], lhsT, rhs,
                                    start=first, stop=last,
                                    tile_position=(p0, q0))
                                if USE_LDW_SKIP and j > 0:
                                    mm.ldweights = False
                for rh in range(2):
                    for j in range(L2):
                        stg = stage_pool.tile([128, 512], dt.bfloat16)
                        nc.scalar.activation(
                            stg, ps[rh][j],
                            mybir.ActivationFunctionType.Gelu,
                            scale=bntile[:, e, 0:1],
                            bias=bntile[:, e, 1:2])
                        nc.sync.dma_start(out=out[:, unit, :], in_=stg)
                        unit += 1
            assert unit == nunits

    nc.compile()
    return nc


def _get_program(key, runseq):
    if key not in _COMPILED:
        _COMPILED[key] = _build_program(runseq)
    return _COMPILED[key]


def _prep_weights(ws, bs, bn_scale, bn_bias, bn_mean, bn_var):
    bf16 = ml_dtypes.bfloat16
    wt = np.empty((CIN, NTAPS, CE), dtype=bf16)
    for e in range(4):
        k = KS[e]
        w = ws[e].astype(np.float32)  # [CE, CIN, k, k]
        wt[:, _SLOT_BASE[e]:_SLOT_BASE[e] + k * k, :] = (
            w.transpose(1, 2, 3, 0).reshape(CIN, k * k, CE).astype(bf16))
    inv = (bn_scale / np.sqrt(bn_var + BN_EPS)).astype(np.float32)
    shift = (np.stack(bs) * inv + bn_bias - bn_mean * inv).astype(np.float32)
    bnp = np.stack([inv, shift], axis=1)              # [4, 2, CE]
    bnp = np.ascontiguousarray(bnp.transpose(2, 0, 1))  # [CE, 4, 2]
    return wt, bnp


def run(inputs, trace=False):
    from concourse import bass_utils

    x = np.asarray(inputs["x"], dtype=np.float32)
    ws = [np.asarray(inputs[f"w{i}"], dtype=np.float32) for i in range(4)]
    bs = [np.asarray(inputs[f"b{i}"], dtype=np.float32) for i in range(4)]
    bn_scale = np.asarray(inputs["bn_scale"], dtype=np.float32)
    bn_bias = np.asarray(inputs["bn_bias"], dtype=np.float32)
    bn_mean = np.asarray(inputs["bn_mean"], dtype=np.float32)
    bn_var = np.asarray(inputs["bn_var"], dtype=np.float32)
    gate_w = np.asarray(inputs["gate_w"], dtype=np.float32)
    gate_b = np.asarray(inputs["gate_b"], dtype=np.float32)

    idx, wsel = _gate(x, gate_w, gate_b)
    key, patterns, runseq, units, assign = _plan(idx)
    nc = _get_program(key, runseq)
    wt, bnp = _prep_weights(ws, bs, bn_scale, bn_bias, bn_mean, bn_var)

    bf16 = ml_dtypes.bfloat16
    xpad = np.zeros((B, CIN, HP, WP), dtype=bf16)
    xpad[:, :, PAD:PAD + H, PAD:PAD + W] = x.astype(bf16)

    totrows = sum(r[3] for r in runseq)
    in_maps = []
    for c in range(NCORES):
        xin = np.empty((2, CIN, totrows, WP), dtype=bf16)
        for rh in range(2):
            i = c * 2 + rh
            pos_ctr = {}
            for e, L, rowoff, rows in runseq:
                pos = pos_ctr.get(e, 0)
                pos_ctr[e] = pos + 1
                s, c0 = assign[(i, e, pos)]
                xin[rh, :, rowoff:rowoff + rows, :] = (
                    xpad[s, :, 8 * c0:8 * c0 + rows, :])
        in_maps.append({"xin0": xin[0], "xin1": xin[1],
                        "wt": wt, "bnp": bnp})

    res = bass_utils.run_bass_kernel_spmd(
        nc, in_maps, core_ids=list(range(NCORES)), trace=trace)

    # scatter device outputs -> (sample, rank) feature maps, weight, concat
    rank = {}
    for s in range(B):
        rank[(s, idx[s, 0])] = 0
        rank[(s, idx[s, 1])] = 1
    E = np.zeros((B, 2, CE, HO, WO), dtype=np.float32)
    # per-(e) position counters replicated per core (same unit order)
    for c in range(NCORES):
        o = res.results[c]["out"]  # [128, nunits, 512] bf16
        of = o.astype(np.float32)
        for u, (e, pos, rh, j, L2) in enumerate(units):
            i = c * 2 + rh
            s, c0 = assign[(i, e, pos)]
            r = rank[(s, e)]
            ca, cb = c0 + j, c0 + L2 + j
            E[s, r, :, 4 * ca:4 * ca + 4, :] = of[0:64, u].reshape(64, 4, WO)
            E[s, r, :, 4 * cb:4 * cb + 4, :] = of[64:128, u].reshape(64, 4, WO)
    outf = (E * wsel[:, :, None, None, None]).reshape(B, 2 * CE, HO, WO)
    return np.ascontiguousarray(outf), res


def kernel(**inputs):
    outf, _ = run(inputs, trace=False)
    return outf


# revision 13
# speedup vs baseline: 1.3555x; 1.0870x over previous
"""MoE downsample kernel for 8 TRN2 NeuronCores — top-2 sparse version.

Host computes the gate (cheap 16x64x4 matvec) first, so the device only
computes each sample's two selected experts (~half the dense MACs). The
SPMD constraint (one program on all 8 cores) is satisfied by an
expert-major uniform schedule parameterized only by the per-expert
selection counts n_e: for expert e every row-half instance (8 cores x 2
PE row-halves = 16 instances) runs an identical pattern of chunk-runs
(run lengths = binary decomposition of 2*n_e into {8,4,2}); the host
gathers the right (sample, chunk-range) input slab into each run's slot
and scatters the outputs back.

Per run of L chunks (chunk = 4 output rows = 512 px): the two PE column
halves process L/2 chunks each, tap-outer loop so one LDWEIGHTS per
(tap, quadrant) covers L/2 matmuls (trailing matmuls set ldweights=False
to reuse the loaded stationary operand). BN + conv-bias + GELU fused
into the ScalarE PSUM eviction; outputs written bf16; gate weighting and
top-2 concat on host.
"""

import numpy as np
import ml_dtypes

KS = [3, 5, 7, 9]
DS = [1, 2, 3, 4]
BN_EPS = 1e-5
B, CIN, H, W = 16, 64, 256, 256
CE = 64
PAD = 16
HP = WP = PAD + 256 + 15   # 287
HO = WO = 128
NCORES = 8
NTAPS = sum(k * k for k in KS)  # 164
NCHUNKS = 32                    # 4 output rows per chunk
EXPERT_ORDER = [3, 2, 0, 1]     # heavy first; light-DMA e1 last
USE_LDW_SKIP = True

_SLOT_BASE = np.cumsum([0] + [k * k for k in KS]).tolist()

# Per-expert slab geometry. Experts with even dilation (e1 d=2, e3 d=4)
# only ever read even rows/cols of the padded image (offsets and strides
# all even), so the host pre-decimates those slabs 2x in each dim.
# ro/co ranges: ro = d*u - pad + PAD over u in [0,k).
_GEOM = {}
for _e in range(4):
    _k, _d = KS[_e], DS[_e]
    _pad = _d * (_k - 1) // 2
    _ro = [_d * _u - _pad + PAD for _u in range(_k)]
    _rs = 2 if _d % 2 == 0 else 1      # host decimation factor
    _ro_min, _ro_max = min(_ro), max(_ro)
    # slab rows for a run of L chunks (in decimated units)
    # original rows [ro_min, 8(L-1)+ro_max+6]
    _GEOM[_e] = {
        "rs": _rs,
        "ro_min": _ro_min,
        "rows": lambda L, a=_ro_min, b=_ro_max, r=_rs: (8 * (L - 1) + b + 6 - a) // r + 1,
        "cols": (_ro_max - _ro_min + 2 * (WO - 1)) // _rs + 1,
    }
_MAXROWS = max(_GEOM[e]["rows"](8) for e in range(4))
_MAXCOLS = max(_GEOM[e]["cols"] for e in range(4))

_COMPILED = {}


def _tap_offsets(e):
    """(slot, row_off, col_off) in padded slab coords for expert e."""
    k, d = KS[e], DS[e]
    pad = d * (k - 1) // 2
    for u in range(k):
        for v in range(k):
            slot = _SLOT_BASE[e] + u * k + v
            yield slot, d * u - pad + PAD, d * v - pad + PAD


def _pattern(two_n):
    """Run lengths (each in {2,4,8}, smallest first) summing to 2*n_e."""
    out = []
    if two_n & 2:
        out.append(2)
    if two_n & 4:
        out.append(4)
    out += [8] * ((two_n - sum(out)) // 8)
    assert sum(out) == two_n
    return out


def _gate(x, gate_w, gate_b):
    pooled = x.astype(np.float64).mean(axis=(2, 3)).astype(np.float32)
    logits = pooled @ gate_w.T.astype(np.float32) + gate_b
    z = logits - logits.max(axis=1, keepdims=True)
    ez = np.exp(z.astype(np.float32))
    gates = ez / ez.sum(axis=1, keepdims=True)
    idx = np.argsort(-gates, axis=1, kind="stable")[:, :2]
    wsel = np.take_along_axis(gates, idx, axis=1)
    wsel = wsel / (wsel.sum(axis=1, keepdims=True) + 1e-8)
    return idx, wsel.astype(np.float32)


def _plan(idx):
    """Build the uniform schedule + per-instance run assignment.

    Returns (key, patterns, runseq, units, assign):
      key      — cache key for the compiled program (depends on n only)
      patterns — {e: [run lengths]}
      runseq   — [(e, L, rowoff, rows)] in program order
      units    — [(e, pos, rh, j, L2)] out-unit order (per core)
      assign   — {(instance 0..15, e, pos): (sample, c0)}
    """
    n = [0, 0, 0, 0]
    samples_e = {e: [] for e in range(4)}
    for s in range(B):
        for e in idx[s]:
            n[e] += 1
            samples_e[e].append(s)
    patterns = {e: _pattern(2 * n[e]) for e in range(4)}
    key = tuple(n)

    # global run inventory per (e, L): cut samples into within-sample runs
    assign = {}
    for e in EXPERT_ORDER:
        pat = patterns[e]
        need = {}
        for L in pat:
            need[L] = need.get(L, 0) + 16
        cursor = 0
        inventory = {L: [] for L in need}
        for L in sorted(need, reverse=True):
            ns = need[L] * L // NCHUNKS     # samples consumed at this L
            assert ns * NCHUNKS == need[L] * L
            for _ in range(ns):
                s = samples_e[e][cursor]
                cursor += 1
                for c0 in range(0, NCHUNKS, L):
                    inventory[L].append((s, c0))
        assert cursor == n[e], (e, cursor, n[e])
        ptr = {L: 0 for L in need}
        for pos, L in enumerate(pat):
            for i in range(16):
                assign[(i, e, pos)] = inventory[L][ptr[L]]
                ptr[L] += 1
        for L in need:
            assert ptr[L] == len(inventory[L])

    runseq = []
    units = []
    rowoff = 0
    for e in EXPERT_ORDER:
        for pos, L in enumerate(patterns[e]):
            rows = _GEOM[e]["rows"](L)
            runseq.append((e, L, rowoff, rows))
            rowoff += rows
            L2 = L // 2
            for rh in range(2):
                for j in range(L2):
                    units.append((e, pos, rh, j, L2))
    return key, patterns, runseq, units, assign


def _build_program(runseq):
    import concourse.bass as bass  # noqa: F401
    import concourse.mybir as mybir
    import concourse.tile as tile
    from concourse import bacc
    from contextlib import ExitStack

    dt = mybir.dt
    totrows = sum(r[3] for r in runseq)
    nunits = sum(2 * (r[1] // 2) for r in runseq)

    nc = bacc.Bacc("TRN2", target_bir_lowering=False, debug=False,
                   num_devices=NCORES)
    xin0 = nc.dram_tensor("xin0", [CIN, totrows, _MAXCOLS], dt.bfloat16,
                          kind="ExternalInput")
    xin1 = nc.dram_tensor("xin1", [CIN, totrows, _MAXCOLS], dt.bfloat16,
                          kind="ExternalInput")
    wt = nc.dram_tensor("wt", [CIN, NTAPS, CE], dt.bfloat16,
                        kind="ExternalInput")
    bnp = nc.dram_tensor("bnp", [CE, 4, 2], dt.float32, kind="ExternalInput")
    out = nc.dram_tensor("out", [128, nunits, 512], dt.bfloat16,
                         kind="ExternalOutput")

    with tile.TileContext(nc) as tc:
        with ExitStack() as ctx:
            consts = ctx.enter_context(tc.tile_pool(name="consts", bufs=1))
            slab_pool = ctx.enter_context(tc.tile_pool(name="slab", bufs=3))
            stage_pool = ctx.enter_context(tc.tile_pool(name="st", bufs=8))
            psum_pool = ctx.enter_context(
                tc.tile_pool(name="ps", bufs=1, space="PSUM"))

            wtile = consts.tile([128, NTAPS, CE], dt.bfloat16)
            bntile = consts.tile([128, 4, 2], dt.float32)
            for half in range(2):
                p0 = half * 64
                nc.gpsimd.dma_start(out=wtile[p0:p0 + 64, :, :], in_=wt.ap())
                nc.gpsimd.dma_start(out=bntile[p0:p0 + 64, :, :], in_=bnp.ap())

            # HAM warmup: ~7us of dummy matmuls while the first slab DMAs.
            warm_ps = psum_pool.tile([128, 512], dt.float32, name="psb_0_0")
            for _ in range(16):
                nc.tensor.matmul(warm_ps[0:64, :], wtile[0:64, 0, :],
                                 wtile[0:64, 0:8, :], start=True, stop=True,
                                 tile_position=(0, 0))

            unit = 0
            for e, L, rowoff, rows in runseq:
                taps = list(_tap_offsets(e))
                T = len(taps)
                L2 = L // 2
                g = _GEOM[e]
                rs, base = g["rs"], g["ro_min"]
                cols = g["cols"]
                st = slab_pool.tile([128, _MAXROWS, _MAXCOLS], dt.bfloat16,
                                    name="st")
                nc.gpsimd.dma_start(out=st[0:64, 0:rows, 0:cols],
                                    in_=xin0[:, rowoff:rowoff + rows, 0:cols])
                nc.gpsimd.dma_start(out=st[64:128, 0:rows, 0:cols],
                                    in_=xin1[:, rowoff:rowoff + rows, 0:cols])
                ps = [[psum_pool.tile([128, 512], dt.float32,
                                      name=f"psb_{rh}_{j}")
                       for j in range(L2)] for rh in range(2)]
                for t, (slot, ro, co) in enumerate(taps):
                    first = t == 0
                    last = t == T - 1
                    # chunk-index outer, quadrant inner: the PE queue is
                    # in-order, so consecutive MMs must hit different
                    # quadrants to keep all four streaming concurrently.
                    for j in range(L2):
                        for rh in range(2):
                            p0 = rh * 64
                            lhsT = wtile[p0:p0 + 64, slot, :]
                            for col in range(2):
                                q0 = col * 64
                                cc = col * L2 + j
                                rl = (8 * cc + ro - base) // rs
                                cl = (co - base) // rs
                                rstep = 2 // rs
                                rhs = st[p0:p0 + 64,
                                         rl:rl + 3 * rstep + 1:rstep,
                                         cl:cl + (WO - 1) * rstep + 1:rstep]
                                mm = nc.tensor.matmul(
                                    ps[rh][j][q0:q0 + 64, :], lhsT, rhs,
                                    start=first, stop=last,
                                    tile_position=(p0, q0))
                                if USE_LDW_SKIP and j > 0:
                                    mm.ldweights = False
                for rh in range(2):
                    for j in range(L2):
                        stg = stage_pool.tile([128, 512], dt.bfloat16)
                        nc.scalar.activation(
                            stg, ps[rh][j],
                            mybir.ActivationFunctionType.Gelu,
                            scale=bntile[:, e, 0:1],
                            bias=bntile[:, e, 1:2])
                        nc.sync.dma_start(out=out[:, unit, :], in_=stg)
                        unit += 1
            assert unit == nunits

    nc.compile()
    return nc


def _get_program(key, runseq):
    if key not in _COMPILED:
        _COMPILED[key] = _build_program(runseq)
    return _COMPILED[key]


def _prep_weights(ws, bs, bn_scale, bn_bias, bn_mean, bn_var):
    bf16 = ml_dtypes.bfloat16
    wt = np.empty((CIN, NTAPS, CE), dtype=bf16)
    for e in range(4):
        k = KS[e]
        w = ws[e].astype(np.float32)  # [CE, CIN, k, k]
        wt[:, _SLOT_BASE[e]:_SLOT_BASE[e] + k * k, :] = (
            w.transpose(1, 2, 3, 0).reshape(CIN, k * k, CE).astype(bf16))
    inv = (bn_scale / np.sqrt(bn_var + BN_EPS)).astype(np.float32)
    shift = (np.stack(bs) * inv + bn_bias - bn_mean * inv).astype(np.float32)
    bnp = np.stack([inv, shift], axis=1)              # [4, 2, CE]
    bnp = np.ascontiguousarray(bnp.transpose(2, 0, 1))  # [CE, 4, 2]
    return wt, bnp


def run(inputs, trace=False):
    from concourse import bass_utils

    x = np.asarray(inputs["x"], dtype=np.float32)
    ws = [np.asarray(inputs[f"w{i}"], dtype=np.float32) for i in range(4)]
    bs = [np.asarray(inputs[f"b{i}"], dtype=np.float32) for i in range(4)]
    bn_scale = np.asarray(inputs["bn_scale"], dtype=np.float32)
    bn_bias = np.asarray(inputs["bn_bias"], dtype=np.float32)
    bn_mean = np.asarray(inputs["bn_mean"], dtype=np.float32)
    bn_var = np.asarray(inputs["bn_var"], dtype=np.float32)
    gate_w = np.asarray(inputs["gate_w"], dtype=np.float32)
    gate_b = np.asarray(inputs["gate_b"], dtype=np.float32)

    idx, wsel = _gate(x, gate_w, gate_b)
    key, patterns, runseq, units, assign = _plan(idx)
    nc = _get_program(key, runseq)
    wt, bnp = _prep_weights(ws, bs, bn_scale, bn_bias, bn_mean, bn_var)

    bf16 = ml_dtypes.bfloat16
    xpad = np.zeros((B, CIN, HP, WP), dtype=bf16)
    xpad[:, :, PAD:PAD + H, PAD:PAD + W] = x.astype(bf16)

    totrows = sum(r[3] for r in runseq)
    in_maps = []
    for c in range(NCORES):
        xin = np.zeros((2, CIN, totrows, _MAXCOLS), dtype=bf16)
        for rh in range(2):
            i = c * 2 + rh
            pos_ctr = {}
            for e, L, rowoff, rows in runseq:
                pos = pos_ctr.get(e, 0)
                pos_ctr[e] = pos + 1
                s, c0 = assign[(i, e, pos)]
                g = _GEOM[e]
                rs, base, cols = g["rs"], g["ro_min"], g["cols"]
                r0 = 8 * c0 + base
                xin[rh, :, rowoff:rowoff + rows, 0:cols] = (
                    xpad[s, :, r0:r0 + rows * rs:rs,
                         base:base + cols * rs:rs])
        in_maps.append({"xin0": xin[0], "xin1": xin[1],
                        "wt": wt, "bnp": bnp})

    res = bass_utils.run_bass_kernel_spmd(
        nc, in_maps, core_ids=list(range(NCORES)), trace=trace)

    # scatter device outputs -> (sample, rank) feature maps, weight, concat
    rank = {}
    for s in range(B):
        rank[(s, idx[s, 0])] = 0
        rank[(s, idx[s, 1])] = 1
    E = np.zeros((B, 2, CE, HO, WO), dtype=np.float32)
    # per-(e) position counters replicated per core (same unit order)
    for c in range(NCORES):
        o = res.results[c]["out"]  # [128, nunits, 512] bf16
        of = o.astype(np.float32)
        for u, (e, pos, rh, j, L2) in enumerate(units):
            i = c * 2 + rh
            s, c0 = assign[(i, e, pos)]
            r = rank[(s, e)]
            ca, cb = c0 + j, c0 + L2 + j
            E[s, r, :, 4 * ca:4 * ca + 4, :] = of[0:64, u].reshape(64, 4, WO)
            E[s, r, :, 4 * cb:4 * cb + 4, :] = of[64:128, u].reshape(64, 4, WO)
    outf = (E * wsel[:, :, None, None, None]).reshape(B, 2 * CE, HO, WO)
    return np.ascontiguousarray(outf), res


def kernel(**inputs):
    outf, _ = run(inputs, trace=False)
    return outf


# revision 19
# speedup vs baseline: 1.7357x; 1.2804x over previous
"""MoE downsample kernel for 8 TRN2 NeuronCores — top-2 sparse version.

Host computes the gate (cheap 16x64x4 matvec) first, so the device only
computes each sample's two selected experts (~half the dense MACs). The
SPMD constraint (one program on all 8 cores) is satisfied by an
expert-major uniform schedule parameterized only by the per-expert
selection counts n_e: for expert e every row-half instance (8 cores x 2
PE row-halves = 16 instances) runs an identical pattern of chunk-runs
(run lengths = binary decomposition of 2*n_e into {8,4,2}); the host
gathers the right (sample, chunk-range) input slab into each run's slot
and scatters the outputs back.

Per run of L chunks (chunk = 4 output rows = 512 px): the two PE column
halves process L/2 chunks each, tap-outer loop so one LDWEIGHTS per
(tap, quadrant) covers L/2 matmuls (trailing matmuls set ldweights=False
to reuse the loaded stationary operand). BN + conv-bias + GELU fused
into the ScalarE PSUM eviction; outputs written bf16; gate weighting and
top-2 concat on host.
"""

import numpy as np
import ml_dtypes

KS = [3, 5, 7, 9]
DS = [1, 2, 3, 4]
BN_EPS = 1e-5
B, CIN, H, W = 16, 64, 256, 256
CE = 64
PAD = 16
HP = WP = PAD + 256 + 15   # 287
HO = WO = 128
NCORES = 8
NTAPS = sum(k * k for k in KS)  # 164
NCHUNKS = 32                    # 4 output rows per chunk
EXPERT_ORDER = [3, 2, 0, 1]     # heavy first; light-DMA e1 last
USE_LDW_SKIP = True

_SLOT_BASE = np.cumsum([0] + [k * k for k in KS]).tolist()

# Per-expert slab geometry. Experts with even dilation (e1 d=2, e3 d=4)
# only ever read even rows/cols of the padded image (offsets and strides
# all even), so the host pre-decimates those slabs 2x in each dim.
# ro/co ranges: ro = d*u - pad + PAD over u in [0,k).
_GEOM = {}
for _e in range(4):
    _k, _d = KS[_e], DS[_e]
    _pad = _d * (_k - 1) // 2
    _ro = [_d * _u - _pad + PAD for _u in range(_k)]
    _rs = 2 if _d % 2 == 0 else 1      # host decimation factor
    _ro_min, _ro_max = min(_ro), max(_ro)
    # slab rows for a run of L chunks (in decimated units)
    # original rows [ro_min, 8(L-1)+ro_max+6]
    _GEOM[_e] = {
        "rs": _rs,
        "ro_min": _ro_min,
        "rows": lambda L, a=_ro_min, b=_ro_max, r=_rs: (8 * (L - 1) + b + 6 - a) // r + 1,
        "cols": (_ro_max - _ro_min + 2 * (WO - 1)) // _rs + 1,
    }
_MAXROWS = max(_GEOM[e]["rows"](8) for e in range(4))
_MAXCOLS = max(_GEOM[e]["cols"] for e in range(4))

_COMPILED = {}


def _tap_offsets(e):
    """(slot, row_off, col_off) in padded slab coords for expert e."""
    k, d = KS[e], DS[e]
    pad = d * (k - 1) // 2
    for u in range(k):
        for v in range(k):
            slot = _SLOT_BASE[e] + u * k + v
            yield slot, d * u - pad + PAD, d * v - pad + PAD


def _pattern(two_n):
    """Run lengths (each in {2,4,8}, smallest first) summing to 2*n_e."""
    out = []
    if two_n & 2:
        out.append(2)
    if two_n & 4:
        out.append(4)
    out += [8] * ((two_n - sum(out)) // 8)
    assert sum(out) == two_n
    return out


def _gate(x, gate_w, gate_b):
    pooled = x.astype(np.float64).mean(axis=(2, 3)).astype(np.float32)
    logits = pooled @ gate_w.T.astype(np.float32) + gate_b
    z = logits - logits.max(axis=1, keepdims=True)
    ez = np.exp(z.astype(np.float32))
    gates = ez / ez.sum(axis=1, keepdims=True)
    idx = np.argsort(-gates, axis=1, kind="stable")[:, :2]
    wsel = np.take_along_axis(gates, idx, axis=1)
    wsel = wsel / (wsel.sum(axis=1, keepdims=True) + 1e-8)
    return idx, wsel.astype(np.float32)


def _plan(idx):
    """Build the uniform schedule + per-instance run assignment.

    Returns (key, patterns, runseq, units, assign):
      key      — cache key for the compiled program (depends on n only)
      patterns — {e: [run lengths]}
      runseq   — [(e, L, rowoff, rows)] in program order
      units    — [(e, pos, rh, j, L2)] out-unit order (per core)
      assign   — {(instance 0..15, e, pos): (sample, c0)}
    """
    n = [0, 0, 0, 0]
    samples_e = {e: [] for e in range(4)}
    for s in range(B):
        for e in idx[s]:
            n[e] += 1
            samples_e[e].append(s)
    patterns = {e: _pattern(2 * n[e]) for e in range(4)}
    key = tuple(n)

    # global run inventory per (e, L): cut samples into within-sample runs
    assign = {}
    for e in EXPERT_ORDER:
        pat = patterns[e]
        need = {}
        for L in pat:
            need[L] = need.get(L, 0) + 16
        cursor = 0
        inventory = {L: [] for L in need}
        for L in sorted(need, reverse=True):
            ns = need[L] * L // NCHUNKS     # samples consumed at this L
            assert ns * NCHUNKS == need[L] * L
            for _ in range(ns):
                s = samples_e[e][cursor]
                cursor += 1
                for c0 in range(0, NCHUNKS, L):
                    inventory[L].append((s, c0))
        assert cursor == n[e], (e, cursor, n[e])
        ptr = {L: 0 for L in need}
        for pos, L in enumerate(pat):
            for i in range(16):
                assign[(i, e, pos)] = inventory[L][ptr[L]]
                ptr[L] += 1
        for L in need:
            assert ptr[L] == len(inventory[L])

    runseq = []
    units = []
    rowoff = 0
    for e in EXPERT_ORDER:
        for pos, L in enumerate(patterns[e]):
            rows = _GEOM[e]["rows"](L)
            runseq.append((e, L, rowoff, rows))
            rowoff += rows
            L2 = L // 2
            for rh in range(2):
                for j in range(L2):
                    units.append((e, pos, rh, j, L2))
    return key, patterns, runseq, units, assign


def _dedup_ldweights(blocks):
    """Drop InstLdweights that reload the stationary operand already in a
    quadrant (same weights AP + tile_position, no intervening load). The
    following matmuls (ldweights=False) then reuse the loaded weights.
    Deps of a dropped load are merged into its paired matmul."""
    ndrop = 0
    for bb, insts in blocks.items():
        last = {}
        keep = []
        i = 0
        while i < len(insts):
            inst = insts[i]
            if type(inst).__name__ == "InstLdweights":
                ap = inst.ins[0]
                tp = str(getattr(inst, "tile_position", None))
                sig = (str(ap), str(getattr(inst, "perf_mode", None)))
                nxt = insts[i + 1] if i + 1 < len(insts) else None
                if (last.get(tp) == sig and nxt is not None
                        and type(nxt).__name__ == "InstMatmult"):
                    nxt.merge_dependencies_from(inst)
                    ndrop += 1
                    i += 1
                    continue
                last[tp] = sig
            keep.append(inst)
            i += 1
        insts[:] = keep
    return ndrop


def _build_program(runseq):
    import concourse.bass as bass  # noqa: F401
    import concourse.mybir as mybir
    import concourse.tile as tile
    from concourse import bacc
    from contextlib import ExitStack

    dt = mybir.dt
    totrows = sum(r[3] for r in runseq)
    nunits = sum(2 * (r[1] // 2) for r in runseq)

    nc = bacc.Bacc("TRN2", target_bir_lowering=False, debug=False,
                   num_devices=NCORES)
    xin0 = nc.dram_tensor("xin0", [CIN, totrows, _MAXCOLS], dt.bfloat16,
                          kind="ExternalInput")
    xin1 = nc.dram_tensor("xin1", [CIN, totrows, _MAXCOLS], dt.bfloat16,
                          kind="ExternalInput")
    wt = nc.dram_tensor("wt", [CIN, NTAPS, CE], dt.bfloat16,
                        kind="ExternalInput")
    bnp = nc.dram_tensor("bnp", [CE, 4, 2], dt.float32, kind="ExternalInput")
    out = nc.dram_tensor("out", [128, nunits, 512], dt.bfloat16,
                         kind="ExternalOutput")

    orig_legalize = tile.tile_legalize

    def legalize_and_dedup(blocks, nc_):
        res = orig_legalize(blocks, nc_)
        _dedup_ldweights(res)
        return res

    tile.tile_legalize = legalize_and_dedup
    try:
        _trace_program(nc, tile, mybir, dt, runseq, xin0, xin1, wt, bnp, out)
    finally:
        tile.tile_legalize = orig_legalize

    nc.compile()
    return nc


def _trace_program(nc, tile, mybir, dt, runseq, xin0, xin1, wt, bnp, out):
    from contextlib import ExitStack

    nunits = sum(2 * (r[1] // 2) for r in runseq)
    with tile.TileContext(nc) as tc:
        with ExitStack() as ctx:
            consts = ctx.enter_context(tc.tile_pool(name="consts", bufs=1))
            slab_pool = ctx.enter_context(tc.tile_pool(name="slab", bufs=4))
            stage_pool = ctx.enter_context(tc.tile_pool(name="st", bufs=8))
            psum_pool = ctx.enter_context(
                tc.tile_pool(name="ps", bufs=1, space="PSUM"))

            wtile = consts.tile([128, NTAPS, CE], dt.bfloat16)
            bntile = consts.tile([128, 4, 2], dt.float32)
            for half in range(2):
                p0 = half * 64
                nc.gpsimd.dma_start(out=wtile[p0:p0 + 64, :, :], in_=wt.ap())
                nc.gpsimd.dma_start(out=bntile[p0:p0 + 64, :, :], in_=bnp.ap())

            # HAM warmup: ~7us of dummy matmuls while the first slab DMAs.
            warm_ps = psum_pool.tile([128, 512], dt.float32, name="psb_0_0")
            for _ in range(16):
                nc.tensor.matmul(warm_ps[0:64, :], wtile[0:64, 0, :],
                                 wtile[0:64, 0:8, :], start=True, stop=True,
                                 tile_position=(0, 0))

            unit = 0
            for e, L, rowoff, rows in runseq:
                taps = list(_tap_offsets(e))
                T = len(taps)
                L2 = L // 2
                g = _GEOM[e]
                rs, base = g["rs"], g["ro_min"]
                cols = g["cols"]
                st = slab_pool.tile([128, _MAXROWS, _MAXCOLS], dt.bfloat16,
                                    name="st")
                nc.gpsimd.dma_start(out=st[0:64, 0:rows, 0:cols],
                                    in_=xin0[:, rowoff:rowoff + rows, 0:cols])
                nc.gpsimd.dma_start(out=st[64:128, 0:rows, 0:cols],
                                    in_=xin1[:, rowoff:rowoff + rows, 0:cols])
                ps = [[psum_pool.tile([128, 512], dt.float32,
                                      name=f"psb_{rh}_{j}")
                       for j in range(L2)] for rh in range(2)]
                for t, (slot, ro, co) in enumerate(taps):
                    first = t == 0
                    last = t == T - 1
                    # chunk-index outer, quadrant inner: the PE queue is
                    # in-order, so consecutive MMs must hit different
                    # quadrants to keep all four streaming concurrently.
                    for j in range(L2):
                        for rh in range(2):
                            p0 = rh * 64
                            lhsT = wtile[p0:p0 + 64, slot, :]
                            for col in range(2):
                                q0 = col * 64
                                cc = col * L2 + j
                                rl = (8 * cc + ro - base) // rs
                                cl = (co - base) // rs
                                rstep = 2 // rs
                                rhs = st[p0:p0 + 64,
                                         rl:rl + 3 * rstep + 1:rstep,
                                         cl:cl + (WO - 1) * rstep + 1:rstep]
                                mm = nc.tensor.matmul(
                                    ps[rh][j][q0:q0 + 64, :], lhsT, rhs,
                                    start=first, stop=last,
                                    tile_position=(p0, q0))
                                if USE_LDW_SKIP and j > 0:
                                    mm.ldweights = False
                for rh in range(2):
                    for j in range(L2):
                        stg = stage_pool.tile([128, 512], dt.bfloat16)
                        nc.scalar.activation(
                            stg, ps[rh][j],
                            mybir.ActivationFunctionType.Gelu,
                            scale=bntile[:, e, 0:1],
                            bias=bntile[:, e, 1:2])
                        nc.sync.dma_start(out=out[:, unit, :], in_=stg)
                        unit += 1
            assert unit == nunits


def _get_program(key, runseq):
    if key not in _COMPILED:
        _COMPILED[key] = _build_program(runseq)
    return _COMPILED[key]


def _prep_weights(ws, bs, bn_scale, bn_bias, bn_mean, bn_var):
    bf16 = ml_dtypes.bfloat16
    wt = np.empty((CIN, NTAPS, CE), dtype=bf16)
    for e in range(4):
        k = KS[e]
        w = ws[e].astype(np.float32)  # [CE, CIN, k, k]
        wt[:, _SLOT_BASE[e]:_SLOT_BASE[e] + k * k, :] = (
            w.transpose(1, 2, 3, 0).reshape(CIN, k * k, CE).astype(bf16))
    inv = (bn_scale / np.sqrt(bn_var + BN_EPS)).astype(np.float32)
    shift = (np.stack(bs) * inv + bn_bias - bn_mean * inv).astype(np.float32)
    bnp = np.stack([inv, shift], axis=1)              # [4, 2, CE]
    bnp = np.ascontiguousarray(bnp.transpose(2, 0, 1))  # [CE, 4, 2]
    return wt, bnp


def run(inputs, trace=False):
    from concourse import bass_utils

    x = np.asarray(inputs["x"], dtype=np.float32)
    ws = [np.asarray(inputs[f"w{i}"], dtype=np.float32) for i in range(4)]
    bs = [np.asarray(inputs[f"b{i}"], dtype=np.float32) for i in range(4)]
    bn_scale = np.asarray(inputs["bn_scale"], dtype=np.float32)
    bn_bias = np.asarray(inputs["bn_bias"], dtype=np.float32)
    bn_mean = np.asarray(inputs["bn_mean"], dtype=np.float32)
    bn_var = np.asarray(inputs["bn_var"], dtype=np.float32)
    gate_w = np.asarray(inputs["gate_w"], dtype=np.float32)
    gate_b = np.asarray(inputs["gate_b"], dtype=np.float32)

    idx, wsel = _gate(x, gate_w, gate_b)
    key, patterns, runseq, units, assign = _plan(idx)
    nc = _get_program(key, runseq)
    wt, bnp = _prep_weights(ws, bs, bn_scale, bn_bias, bn_mean, bn_var)

    bf16 = ml_dtypes.bfloat16
    xpad = np.zeros((B, CIN, HP, WP), dtype=bf16)
    xpad[:, :, PAD:PAD + H, PAD:PAD + W] = x.astype(bf16)

    totrows = sum(r[3] for r in runseq)
    in_maps = []
    for c in range(NCORES):
        xin = np.zeros((2, CIN, totrows, _MAXCOLS), dtype=bf16)
        for rh in range(2):
            i = c * 2 + rh
            pos_ctr = {}
            for e, L, rowoff, rows in runseq:
                pos = pos_ctr.get(e, 0)
                pos_ctr[e] = pos + 1
                s, c0 = assign[(i, e, pos)]
                g = _GEOM[e]
                rs, base, cols = g["rs"], g["ro_min"], g["cols"]
                r0 = 8 * c0 + base
                xin[rh, :, rowoff:rowoff + rows, 0:cols] = (
                    xpad[s, :, r0:r0 + rows * rs:rs,
                         base:base + cols * rs:rs])
        in_maps.append({"xin0": xin[0], "xin1": xin[1],
                        "wt": wt, "bnp": bnp})

    res = bass_utils.run_bass_kernel_spmd(
        nc, in_maps, core_ids=list(range(NCORES)), trace=trace)

    # scatter device outputs -> (sample, rank) feature maps, weight, concat
    rank = {}
    for s in range(B):
        rank[(s, idx[s, 0])] = 0
        rank[(s, idx[s, 1])] = 1
    E = np.zeros((B, 2, CE, HO, WO), dtype=np.float32)
    # per-(e) position counters replicated per core (same unit order)
    for c in range(NCORES):
        o = res.results[c]["out"]  # [128, nunits, 512] bf16
        of = o.astype(np.float32)
        for u, (e, pos, rh, j, L2) in enumerate(units):
            i = c * 2 + rh
            s, c0 = assign[(i, e, pos)]
            r = rank[(s, e)]
            ca, cb = c0 + j, c0 + L2 + j
            E[s, r, :, 4 * ca:4 * ca + 4, :] = of[0:64, u].reshape(64, 4, WO)
            E[s, r, :, 4 * cb:4 * cb + 4, :] = of[64:128, u].reshape(64, 4, WO)
    outf = (E * wsel[:, :, None, None, None]).reshape(B, 2 * CE, HO, WO)
    return np.ascontiguousarray(outf), res


def kernel(**inputs):
    outf, _ = run(inputs, trace=False)
    return outf
